# revision 1
# baseline (speedup 1.0000x reference)
"""Trainium2 Bass kernel for nn_DMGAGRUcell (GRU cell with graph-conv gates).

Math (per batch b):
  x    = [inputs | hx]                      (N, 66)
  x1   = S @ x, x2 = adp[b] @ x             (diffusion + adaptive hop)
  ru   = sigmoid([x|x1|x2]_interleaved @ W_ru);  r, u = split(ru)
  c    = tanh([x|x1|x2']_interleaved @ W_c)  with x' = [inputs | r*hx]
  out  = u*hx + (1-u)*c

Sharding: 2 batches per core x 8 cores (data parallel over B=16).
Device layout is feature-major (transposed): all gconv outputs are computed
as x1T = x.T @ S.T etc. with the small x as the PE stationary operand and the
big matrix streaming; adp[b] (bf16, host-pre-transposed) stays resident in
SBUF so HBM reads it once per batch. The dominant-magnitude gate chunks
(x0 @ W0, rh @ W) run in fp32; the small x1/x2 chunks run in bf16.
"""

import os
import numpy as np
import ml_dtypes

BF16 = ml_dtypes.bfloat16

N = 2048
B = 16
D_IN = 2
UNITS = 64
F = 66
B_LOC = 2          # batches per core
N_CORES = 8
KC = 16            # k chunks of 128 nodes
NS = 4             # 512-wide output slabs

_CACHE = {}


def _build():
    if "nc" in _CACHE:
        return _CACHE["nc"]

    from contextlib import ExitStack
    import concourse.mybir as mybir
    import concourse.tile as tile
    from concourse import bacc

    f32 = mybir.dt.float32
    bf = mybir.dt.bfloat16
    f8 = mybir.dt.float8e4
    AF = mybir.ActivationFunctionType

    nc = bacc.Bacc("TRN2", target_bir_lowering=False, debug=False,
                   num_devices=N_CORES)

    adpT_d = nc.dram_tensor("adpT", [B_LOC, KC, 128, N], f8, kind="ExternalInput")
    sT_d = nc.dram_tensor("sT", [KC, 128, N], bf, kind="ExternalInput")
    xnm_d = nc.dram_tensor("xnm", [B_LOC, 128, KC, F], bf, kind="ExternalInput")
    x0Tb_d = nc.dram_tensor("x0Tb", [B_LOC, F, N], bf, kind="ExternalInput")
    hxTf_d = nc.dram_tensor("hxTf", [B_LOC, UNITS, N], f32, kind="ExternalInput")
    wru0_d = nc.dram_tensor("wru0b", [F, 128], bf, kind="ExternalInput")
    wru1_d = nc.dram_tensor("wru1b", [F, 128], bf, kind="ExternalInput")
    wru2_d = nc.dram_tensor("wru2b", [F, 128], bf, kind="ExternalInput")
    wcinp_d = nc.dram_tensor("wcinpb", [D_IN, UNITS], bf, kind="ExternalInput")
    wcrh_d = nc.dram_tensor("wcrhb", [UNITS, UNITS], bf, kind="ExternalInput")
    wc1_d = nc.dram_tensor("wc1b", [F, UNITS], bf, kind="ExternalInput")
    wc2_d = nc.dram_tensor("wc2b", [F, UNITS], bf, kind="ExternalInput")
    id_d = nc.dram_tensor("ident", [UNITS, UNITS], bf, kind="ExternalInput")
    out_d = nc.dram_tensor("outT", [B_LOC, UNITS, N], f32, kind="ExternalOutput")

    with tile.TileContext(nc) as tc, ExitStack() as ctx:
        spool = ctx.enter_context(tc.tile_pool(name="spool", bufs=1))
        apool = ctx.enter_context(tc.tile_pool(name="apool", bufs=16))
        cpool = ctx.enter_context(tc.tile_pool(name="cpool", bufs=1))
        wpool = ctx.enter_context(tc.tile_pool(name="wpool", bufs=1))
        w2pool = ctx.enter_context(tc.tile_pool(name="w2pool", bufs=2))
        pp = ctx.enter_context(tc.tile_pool(name="pp", bufs=8, space="PSUM"))

        # DMA order tracks first use: xnm0, s0 (the first matmuls' inputs),
        # xnm1, the rest of the S stream, then the late-needed inputs
        binp = {}
        xnm0 = w2pool.tile([128, KC, F], bf, tag="xnm", name="xnm0")
        nc.sync.dma_start(xnm0[:], xnm_d[0])
        s_tiles = [spool.tile([128, N], bf, tag="s0", name="s0")]
        for q in range(NS):
            qsl = slice(q * 512, (q + 1) * 512)
            nc.sync.dma_start(s_tiles[0][:, qsl], sT_d[0][:, qsl])
        xnm1 = w2pool.tile([128, KC, F], bf, tag="xnm", name="xnm1")
        nc.sync.dma_start(xnm1[:], xnm_d[1])
        for k in range(1, KC):
            t = spool.tile([128, N], bf, tag=f"s{k}", name=f"s{k}")
            nc.sync.dma_start(t[:], sT_d[k])
            s_tiles.append(t)
        binp[0] = [xnm0]
        binp[1] = [xnm1]
        for b in range(B_LOC):
            x0Tb = w2pool.tile([F, N], bf, tag="x0Tb", name=f"x0Tb{b}")
            nc.sync.dma_start(x0Tb[:], x0Tb_d[b])
            hxTf = wpool.tile([UNITS, N], f32, tag="hxTf", name=f"hxTf{b}")
            nc.sync.dma_start(hxTf[:], hxTf_d[b])
            binp[b] += [x0Tb, hxTf]

        def const(name, dram, shape, dt):
            t = cpool.tile(shape, dt, tag=name, name=name)
            nc.sync.dma_start(t[:], dram[:])
            return t

        wru0 = const("wru0", wru0_d, [F, 128], bf)
        wru1 = const("wru1", wru1_d, [F, 128], bf)
        wru2 = const("wru2", wru2_d, [F, 128], bf)
        wcinp = const("wcinp", wcinp_d, [D_IN, UNITS], bf)
        wcrh = const("wcrh", wcrh_d, [UNITS, UNITS], bf)
        wc1 = const("wc1", wc1_d, [F, UNITS], bf)
        wc2 = const("wc2", wc2_d, [F, UNITS], bf)
        ident = const("ident", id_d, [UNITS, UNITS], bf)

        # warm the ACT function table off the critical path (a function-set
        # switch mid-kernel costs ~1.3us)
        dum = cpool.tile([1, 2], f32, tag="dum", name="dum")
        nc.scalar.activation(dum[0:1, 0:1], ident[0:1, 0:1], AF.Sigmoid)
        nc.scalar.activation(dum[0:1, 1:2], ident[0:1, 0:1], AF.Tanh)

        ADP_SCALE = 1.0 / 2048.0

        def stream_pass(lhs_xnms, rhs_tiles, dsts, pfx, defer_drain=False,
                        scale=None):
            # dsts[i] = lhs_xnms[i].T @ rhs_tiles.T, k-major so several
            # batches' matmuls interleave behind one streamed rhs.
            # Drains stay off the ACT engine: an activation-function switch
            # costs a ~1.3us LoadActFuncSet, so ACT runs only sigmoid/tanh.
            nb = len(lhs_xnms)
            ps = [[pp.tile([F, 512], f32, tag="ps", name=f"ps_{pfx}_{i}_{s}")
                   for s in range(NS)] for i in range(nb)]
            for k in range(KC):
                for i in range(nb):
                    lhsT = lhs_xnms[i][:, k, :]
                    for s in range(NS):
                        nc.tensor.matmul(
                            ps[i][s][:], lhsT,
                            rhs_tiles[k][:, s * 512:(s + 1) * 512],
                            start=(k == 0), stop=(k == KC - 1))
            if defer_drain:
                return ps
            for i in range(nb):
                for s in range(NS):
                    dsl = dsts[i][:, s * 512:(s + 1) * 512]
                    if scale is None:
                        nc.vector.tensor_copy(dsl, ps[i][s][:])
                    else:
                        nc.vector.tensor_scalar_mul(dsl, ps[i][s][:], scale)

        # ---- gconv 1 S-passes, both batches fused behind one S stream ----
        x1Ts = [w2pool.tile([F, N], bf, tag="x1T", name=f"x1T{b}")
                for b in range(B_LOC)]
        stream_pass([binp[0][0], binp[1][0]], s_tiles, x1Ts, "s1")

        for b in range(B_LOC):
            xnm, x0Tb, hxTf = binp[b]

            a_tiles = []
            for k in range(KC):
                t = apool.tile([128, N], f8, tag="adp", name=f"adp_{b}_{k}")
                nc.sync.dma_start(t[:], adpT_d[b, k])
                a_tiles.append(t)

            # ---- gconv 1 adp-pass ----
            x1T = x1Ts[b]
            x2T = w2pool.tile([F, N], bf, tag="x2T")
            stream_pass([xnm], a_tiles, [x2T], f"a1_{b}", scale=ADP_SCALE)

            # ru = sigmoid(x0.W0 + x1.W1 + x2.W2). r and u are computed as
            # separate accumulation groups (W free-dim split) so both land at
            # partitions 0-63 - two-input DVE ops need equal base partitions.
            # ru = sigmoid([x0|x1|x2] @ W_ru): one M=128 accumulation group
            # per slab; r (rows 0-63) and u (rows 64-127) drain via separate
            # sigmoids, u with a shifted partition base down to 0-63.
            # rh = r*hx follows per slab on the DVE; the PE transposes of rh
            # run after all ru matmuls so their input chain is already done.
            ract = wpool.tile([UNITS, N], f32, tag="ract")
            uact = wpool.tile([UNITS, N], f32, tag="uact")
            rhb = wpool.tile([UNITS, N], bf, tag="rhb")
            ru_ps = []
            for s in range(NS):
                sl = slice(s * 512, (s + 1) * 512)
                ps = pp.tile([128, 512], f32, tag="ps", name=f"ps_ru{s}")
                nc.tensor.matmul(ps[:], wru0[:], x0Tb[:, sl], start=True, stop=False)
                nc.tensor.matmul(ps[:], wru1[:], x1T[:, sl], start=False, stop=False)
                nc.tensor.matmul(ps[:], wru2[:], x2T[:, sl], start=False, stop=True)
                nc.scalar.activation(ract[:, sl], ps[0:UNITS, :], AF.Sigmoid)
                nc.vector.tensor_mul(rhb[:, sl], ract[:, sl], hxTf[:, sl])
                ru_ps.append(ps)
            for s in range(NS):
                # u is needed only at the final combine; keep it off the
                # r -> rh -> transpose critical path
                sl = slice(s * 512, (s + 1) * 512)
                nc.scalar.activation(uact[:, sl], ru_ps[s][UNITS:128, :], AF.Sigmoid)
            for k in range(KC):
                pst = pp.tile([128, 1024], bf, tag="ps", name=f"pst_{k}")
                nc.tensor.transpose(
                    pst[:, 0:UNITS], rhb[:, k * 128:(k + 1) * 128], ident[:])
                nc.vector.tensor_copy(xnm[:, k, D_IN:F], pst[:, 0:UNITS])

            # ---- gconv 2 ----
            x1p = w2pool.tile([F, N], bf, tag="x1T")
            x2p = w2pool.tile([F, N], bf, tag="x2T")
            ps1 = stream_pass([xnm], s_tiles, None, f"s2_{b}", defer_drain=True)
            ps2 = stream_pass([xnm], a_tiles, None, f"a2_{b}", defer_drain=True)
            for s in range(NS):
                dsl = slice(s * 512, (s + 1) * 512)
                nc.vector.tensor_copy(x1p[:, dsl], ps1[0][s][:])
                nc.vector.tensor_scalar_mul(x2p[:, dsl], ps2[0][s][:], ADP_SCALE)

            # c = tanh(inp.Wc[0:2] + rh.Wc[2:66] + x1'.Wc1 + x2'.Wc2)
            cT = wpool.tile([UNITS, N], f32, tag="cT")
            outT = wpool.tile([UNITS, N], f32, tag="outT")
            for s in range(NS):
                sl = slice(s * 512, (s + 1) * 512)
                ps = pp.tile([UNITS, 512], f32, tag="ps", name=f"ps_c{s}")
                nc.tensor.matmul(ps[:], wcinp[:], x0Tb[0:D_IN, sl], start=True, stop=False)
                nc.tensor.matmul(ps[:], wcrh[:], rhb[:, sl], start=False, stop=False)
                nc.tensor.matmul(ps[:], wc1[:], x1p[:, sl], start=False, stop=False)
                nc.tensor.matmul(ps[:], wc2[:], x2p[:, sl], start=False, stop=True)
                nc.scalar.activation(cT[:, sl], ps[:], AF.Tanh)
                # out = c + u*(hx - c); alternate slabs between DVE and
                # GpSimd so two dependency chains run in parallel
                eng = nc.vector if s % 2 == 1 else nc.gpsimd
                eng.tensor_sub(outT[:, sl], hxTf[:, sl], cT[:, sl])
                eng.tensor_mul(outT[:, sl], uact[:, sl], outT[:, sl])
                eng.tensor_add(outT[:, sl], outT[:, sl], cT[:, sl])
            nc.sync.dma_start(out_d[b], outT[:])

    nc.compile()
    _CACHE["nc"] = nc
    return nc


def _prep_host(inputs, hx, adp, support_rows, support_cols, support_vals,
               W_ru, W_c):
    xcat = np.concatenate(
        [inputs.reshape(B, N, D_IN), hx.reshape(B, N, UNITS)], axis=2)
    xcat = np.ascontiguousarray(xcat, dtype=np.float32)

    S = np.zeros((N, N), np.float32)
    np.add.at(S, (support_rows, support_cols), support_vals)
    sT = np.ascontiguousarray(S.T).astype(BF16).reshape(KC, 128, N)

    FP8 = ml_dtypes.float8_e4m3fn
    adpT = (np.ascontiguousarray(adp.transpose(0, 2, 1)) * 2048.0).astype(
        FP8).reshape(B, KC, 128, N)

    xnm = xcat.astype(BF16).reshape(B, KC, 128, F).transpose(0, 2, 1, 3)
    xnm = np.ascontiguousarray(xnm)
    x0T = np.ascontiguousarray(xcat.transpose(0, 2, 1))
    x0Tb = x0T.astype(BF16)
    hxTf = np.ascontiguousarray(x0T[:, D_IN:F])

    wru = {
        "wru0b": np.ascontiguousarray(W_ru[0::3]).astype(BF16),
        "wru1b": np.ascontiguousarray(W_ru[1::3]).astype(BF16),
        "wru2b": np.ascontiguousarray(W_ru[2::3]).astype(BF16),
    }
    wc0 = np.ascontiguousarray(W_c[0::3])
    wcd = {
        "wcinpb": np.ascontiguousarray(wc0[0:D_IN]).astype(BF16),
        "wcrhb": np.ascontiguousarray(wc0[D_IN:F]).astype(BF16),
        "wc1b": np.ascontiguousarray(W_c[1::3]).astype(BF16),
        "wc2b": np.ascontiguousarray(W_c[2::3]).astype(BF16),
    }
    ident = np.eye(UNITS, dtype=BF16)

    shared = {"sT": sT, "ident": ident, **wru, **wcd}
    in_maps = []
    for c in range(N_CORES):
        lo, hi = c * B_LOC, (c + 1) * B_LOC
        in_maps.append({
            "adpT": np.ascontiguousarray(adpT[lo:hi]),
            "xnm": np.ascontiguousarray(xnm[lo:hi]),
            "x0Tb": np.ascontiguousarray(x0Tb[lo:hi]),
            "hxTf": np.ascontiguousarray(hxTf[lo:hi]),
            **shared,
        })
    return in_maps


def kernel(inputs, hx, adp, support_rows, support_cols, support_vals,
           W_ru, W_c, time_axis=None):
    from concourse.bass_utils import run_bass_kernel_spmd

    inputs = np.asarray(inputs, dtype=np.float32)
    hx = np.asarray(hx, dtype=np.float32)
    adp = np.asarray(adp, dtype=np.float32)
    support_rows = np.asarray(support_rows)
    support_cols = np.asarray(support_cols)
    support_vals = np.asarray(support_vals, dtype=np.float32)
    W_ru = np.asarray(W_ru, dtype=np.float32)
    W_c = np.asarray(W_c, dtype=np.float32)

    nc = _build()
    in_maps = _prep_host(inputs, hx, adp, support_rows, support_cols,
                         support_vals, W_ru, W_c)

    res = run_bass_kernel_spmd(nc, in_maps, core_ids=list(range(N_CORES)),
                               trace=False)
    _CACHE["last_result"] = res

    out = np.empty((B, N * UNITS), np.float32)
    for c in range(N_CORES):
        outT = res.results[c]["outT"]  # (B_LOC, 64, N)
        for i in range(B_LOC):
            out[c * B_LOC + i] = np.ascontiguousarray(
                outT[i].T).reshape(N * UNITS)
    return out



# revision 7
# speedup vs baseline: 2.1365x; 2.1365x over previous
"""Trainium2 Bass kernel for nn_DMGAGRUcell (GRU cell with graph-conv gates).

Math (per batch b):
  x    = [inputs | hx]                      (N, 66)
  ru   = sigmoid(x W0 + (S x) W1 + (adp x) W2);  r, u = split(ru)
  x'   = [inputs | r*hx]
  c    = tanh(x' Wc0 + (S x') Wc1 + (adp x') Wc2)
  out  = u*hx + (1-u)*c

Sharding: 2 batches per core x 8 cores (data parallel over B=16).

Device strategy:
  - All four N x N streaming products run as fp8e4 DoubleRow matmuls
    (0.5 cycles/row, 2 k-chunks per instruction).  S is scaled x256 and
    adp x32768 so fp8 values stay out of the subnormal range; the scales
    are folded into the gate weights on the host.
  - gconv1 is classic: streams produce x1T/x2T (bf16, feature-major),
    small matmuls per 512-slab accumulate the ru pre-activation.
  - gconv2 is weight-folded: y1 = x'(Wc1*LAM/256), y2 = x'(Wc2*LAM/32768)
    are computed node-major (tiny matmuls, inp-part + rh-part) and
    quantized to fp8; the S/adp streams then accumulate LAM*c_pre
    directly in PSUM (with the Wc0*LAM direct term), and tanh applies
    scale 1/LAM.  No second-gconv drains, no transposes.
  - hx lives at partitions 0:64 and the 2 input rows at 64:66 of one
    packed tile (HW requires 32-aligned partition bases); the matching
    weight rows sit at the same bases so matmul base-pair checks pass.
  - PSUM: two 4-slot rings whose allocation order matches the pass
    windows, so DMA-paced passes never block compute-paced ones.
"""

import numpy as np
import ml_dtypes

BF16 = ml_dtypes.bfloat16
FP8 = ml_dtypes.float8_e4m3fn

N = 2048
B = 16
D_IN = 2
UNITS = 64
F = 66
B_LOC = 2          # batches per core
N_CORES = 8
KC = 16            # k chunks of 128 nodes
KP = 8             # k-chunk pairs (DoubleRow)
NS = 4             # 512-wide output slabs
FPAD = 80          # padded feature count (DoubleRow needs step % 16 == 0)

S_SCALE = 256.0    # fp8 scale for the sparse support matrix
A_SCALE = 32768.0  # fp8 scale for the adaptive adjacency (2048 * 16)
LAM = 32768.0      # common fixed-point scale of the gconv2 PSUM accumulation

_CACHE = {}


def _build():
    if "nc" in _CACHE:
        return _CACHE["nc"]

    from contextlib import ExitStack
    import concourse.mybir as mybir
    import concourse.tile as tile
    from concourse import bacc

    f32 = mybir.dt.float32
    bf = mybir.dt.bfloat16
    f8 = mybir.dt.float8e4
    AF = mybir.ActivationFunctionType
    DR = mybir.MatmulPerfMode.DoubleRow

    nc = bacc.Bacc("TRN2", target_bir_lowering=False, debug=False,
                   num_devices=N_CORES)

    adp_d = nc.dram_tensor("adpT", [B_LOC, KP, 128, 2, N], f8, kind="ExternalInput")
    s_d = nc.dram_tensor("sT", [KP, 128, 2, N], f8, kind="ExternalInput")
    xnm_d = nc.dram_tensor("xnm", [128, B_LOC, KC, FPAD], f8, kind="ExternalInput")
    xb_d = nc.dram_tensor("xb", [B_LOC, F, N], bf, kind="ExternalInput")
    # hxi: rows 0:64 = hx (feature-major), rows 64:66 = inputs
    hxi_d = nc.dram_tensor("hxi", [B_LOC, F, N], bf, kind="ExternalInput")
    # wblob cols: [0:384] wru (3x128), [384:512] wcy (2x64), [512:576] wc0;
    # within wcy/wc0 cols, rows 0:64 are the hx-part, rows 64:66 the inp-part.
    wb_d = nc.dram_tensor("wblob", [F, 576], bf, kind="ExternalInput")
    out_d = nc.dram_tensor("outT", [B_LOC, UNITS, N], bf, kind="ExternalOutput")

    with tile.TileContext(nc) as tc, ExitStack() as ctx:
        cpool = ctx.enter_context(tc.tile_pool(name="cpool", bufs=1))
        spool = ctx.enter_context(tc.tile_pool(name="spool", bufs=1))
        apool = ctx.enter_context(tc.tile_pool(name="apool", bufs=1))
        wk = ctx.enter_context(tc.tile_pool(name="wk", bufs=1))
        pp = ctx.enter_context(tc.tile_pool(name="pp", bufs=4, space="PSUM"))

        def sl(s):
            return slice(s * 512, (s + 1) * 512)

        # ---- tiles (SBUF residency) ----
        xnm = wk.tile([128, B_LOC, KC, FPAD], f8, tag="xnm", name="xnm")
        xb = [wk.tile([F, N], bf, tag=f"xb{b}", name=f"xb{b}")
              for b in range(B_LOC)]
        hxi = [wk.tile([F, N], bf, tag=f"hxi{b}", name=f"hxi{b}")
               for b in range(B_LOC)]
        s2 = [spool.tile([128, 2, N], f8, tag=f"s{kp}", name=f"s{kp}")
              for kp in range(KP)]
        a2 = [[apool.tile([128, 2, N], f8, tag=f"a{b}_{kp}", name=f"a{b}_{kp}")
               for kp in range(KP)] for b in range(B_LOC)]
        wb = cpool.tile([F, 576], bf, tag="wb", name="wb")

        def wru(m):
            return wb[:, 128 * m:128 * (m + 1)]

        def wcyR(j):
            return wb[0:UNITS, 384 + 64 * j:384 + 64 * (j + 1)]

        def wcyI(j):
            return wb[UNITS:F, 384 + 64 * j:384 + 64 * (j + 1)]

        wc0R = wb[0:UNITS, 512:576]
        wc0I = wb[UNITS:F, 512:576]

        x1T = [wk.tile([F, N], bf, tag=f"x1T{b}", name=f"x1T{b}") for b in range(B_LOC)]
        x2T = [wk.tile([F, N], bf, tag=f"x2T{b}", name=f"x2T{b}") for b in range(B_LOC)]
        rhT = [wk.tile([UNITS, N], bf, tag=f"rhT{b}", name=f"rhT{b}") for b in range(B_LOC)]
        ract = [wk.tile([UNITS, N], bf, tag=f"ract{b}", name=f"ract{b}") for b in range(B_LOC)]
        uact = [wk.tile([UNITS, N], bf, tag=f"uact{b}", name=f"uact{b}") for b in range(B_LOC)]
        cT = [wk.tile([UNITS, N], bf, tag=f"cT{b}", name=f"cT{b}") for b in range(B_LOC)]
        outT = [wk.tile([UNITS, N], bf, tag=f"outT{b}", name=f"outT{b}") for b in range(B_LOC)]
        y1nm = [wk.tile([128, KC, UNITS], f8, tag=f"y1nm{b}", name=f"y1nm{b}")
                for b in range(B_LOC)]
        y2nm = [wk.tile([128, KC, UNITS], f8, tag=f"y2nm{b}", name=f"y2nm{b}")
                for b in range(B_LOC)]

        # ---- DMA issue order == HBM arrival order (one serialized bus,
        # ~0.65us HWDGE serialization per DMA, so smalls are packed).
        nc.sync.dma_start(wb[:], wb_d[:])
        nc.sync.dma_start(xnm[:], xnm_d[:])
        for kp in range(KP):
            nc.sync.dma_start(s2[kp][:], s_d[kp])
        nc.sync.dma_start(xb[0][:], xb_d[0])
        nc.sync.dma_start(hxi[0][:], hxi_d[0])
        for kp in range(KP):
            nc.sync.dma_start(a2[0][kp][:], adp_d[0, kp])
        nc.sync.dma_start(xb[1][:], xb_d[1])
        nc.sync.dma_start(hxi[1][:], hxi_d[1])
        for kp in range(KP):
            nc.sync.dma_start(a2[1][kp][:], adp_d[1, kp])

        def drain(dst, src, s):
            # PSUM -> SBUF copies: only DVE and ACT may read PSUM.
            if s % 2 == 0:
                nc.vector.tensor_copy(dst, src)
            else:
                nc.scalar.activation(dst, src, AF.Copy)

        def dr_mm(ps, lhsT_tile, rhs_tile, s, start, stop):
            nc.tensor.matmul(ps[:], lhsT_tile, rhs_tile[:, :, sl(s)],
                             start=start, stop=stop, perf_mode=DR)

        def xnm_pair(b, kp):
            return xnm[:, b, 2 * kp:2 * kp + 2, :]

        def ymm(b, psY, chunks, which):
            # y{1,2} = x' @ Wcy_{1,2}, node-major: per 128-node chunk,
            # two tiny matmuls (inp rows + rh rows) accumulate [128, 64].
            for k in chunks:
                ck = slice(128 * k, 128 * (k + 1))
                dst = psY[k // 8][:, k % 8, :]
                nc.tensor.matmul(dst, hxi[b][UNITS:F, ck], wcyI(which),
                                 start=True, stop=False)
                nc.tensor.matmul(dst, rhT[b][:, ck], wcyR(which),
                                 start=False, stop=True)

        def ynm_copies(b, psY1, psY2):
            # quantize y1/y2 to fp8 node-major, 4-chunk granularity; all of
            # y1 first (the S stream consumes it first).
            for yd, ps in ((y1nm[b], psY1), (y2nm[b], psY2)):
                for q in range(4):
                    src = ps[q // 2][:, 4 * (q % 2):4 * (q % 2) + 4, :]
                    dst = yd[:, 4 * q:4 * q + 4, :]
                    if q % 2 == 0:
                        nc.vector.tensor_copy(dst, src)
                    else:
                        nc.scalar.activation(dst, src, AF.Copy)

        # ============ gconv1 S passes, both batches pair-paced ============
        psS0 = [pp.tile([FPAD, 512], f32, tag="pA", name=f"psS0_{s}")
                for s in range(NS)]
        psS1 = [pp.tile([FPAD, 512], f32, tag="pB", name=f"psS1_{s}")
                for s in range(NS)]
        for kp in range(KP):
            for s in range(NS):
                dr_mm(psS0[s], xnm_pair(0, kp), s2[kp], s, kp == 0, kp == KP - 1)
            for s in range(NS):
                dr_mm(psS1[s], xnm_pair(1, kp), s2[kp], s, kp == 0, kp == KP - 1)
        for s in range(NS):
            drain(x1T[0][:, sl(s)], psS0[s][0:F, :], s)
        for s in range(NS):
            drain(x1T[1][:, sl(s)], psS1[s][0:F, :], s + 1)

        # ru b0 opens with the x0/x1 terms (ring B, freed by sigmoids).
        ru0 = [pp.tile([128, 512], f32, tag="pB", name=f"ru0_{s}")
               for s in range(NS)]
        for s in range(NS):
            nc.tensor.matmul(ru0[s][:], wru(0), xb[0][:, sl(s)],
                             start=True, stop=False)
            nc.tensor.matmul(ru0[s][:], wru(1), x1T[0][:, sl(s)],
                             start=False, stop=False)

        # ============ gconv1 adp pass b0 (pair-paced, ring A) ============
        psA0 = [pp.tile([FPAD, 512], f32, tag="pA", name=f"psA0_{s}")
                for s in range(NS)]
        for kp in range(KP):
            for s in range(NS):
                dr_mm(psA0[s], xnm_pair(0, kp), a2[0][kp], s, kp == 0, kp == KP - 1)

        # Per-slab: drain x2T -> close ru -> sigmoid(r) -> rh, pipelined so
        # slab 0's chain starts as soon as the last adp pair lands.
        for s in range(NS):
            drain(x2T[0][:, sl(s)], psA0[s][0:F, :], s)
            nc.tensor.matmul(ru0[s][:], wru(2), x2T[0][:, sl(s)],
                             start=False, stop=True)
            nc.scalar.activation(ract[0][:, sl(s)], ru0[s][0:UNITS, :],
                                 AF.Sigmoid)
            nc.vector.tensor_mul(rhT[0][:, sl(s)], ract[0][:, sl(s)],
                                 hxi[0][0:UNITS, sl(s)])
            nc.scalar.activation(uact[0][:, sl(s)], ru0[s][UNITS:128, :],
                                 AF.Sigmoid)

        # b1 gconv1 adp pass (ring A, slots freed per-slab by x2T0 drains;
        # pairs 0..4 arrive during the b0 gate chain).
        psA1 = [pp.tile([FPAD, 512], f32, tag="pA", name=f"psA1_{s}")
                for s in range(NS)]

        def a1_block(kp):
            for s in range(NS):
                dr_mm(psA1[s], xnm_pair(1, kp), a2[1][kp], s,
                      kp == 0, kp == KP - 1)

        a1_block(0)
        a1_block(1)
        a1_block(2)

        # y1/y2 for b0 (ring B after ru0 slots release via sigmoids).
        psY0 = [pp.tile([128, 8, UNITS], f32, tag="pB", name=f"psY0_{i}")
                for i in range(4)]
        ymm(0, psY0[0:2], range(KC), 0)
        a1_block(3)
        ymm(0, psY0[2:4], range(KC), 1)
        ynm_copies(0, psY0[0:2], psY0[2:4])
        a1_block(4)

        # ====== b0 gconv2 streams (ring B after psY0) ======
        psC0 = [pp.tile([UNITS, 512], f32, tag="pB", name=f"psC0_{s}")
                for s in range(NS)]
        for s in range(NS):
            nc.tensor.matmul(psC0[s][:], wc0I, hxi[0][UNITS:F, sl(s)],
                             start=True, stop=False)
            nc.tensor.matmul(psC0[s][:], wc0R, rhT[0][:, sl(s)],
                             start=False, stop=False)
        for kp in range(KP):
            for s in range(NS):
                dr_mm(psC0[s], y1nm[0][:, 2 * kp:2 * kp + 2, :], s2[kp], s,
                      False, False)
            if kp == 2:
                a1_block(5)
            if kp == 5:
                a1_block(6)
        # adp section slab-major so tanh/final/store pipeline per slab.
        for s in range(NS):
            for kp in range(KP):
                dr_mm(psC0[s], y2nm[0][:, 2 * kp:2 * kp + 2, :], a2[0][kp], s,
                      False, kp == KP - 1)
            nc.scalar.activation(cT[0][:, sl(s)], psC0[s][:],
                                 AF.Tanh, scale=1.0 / LAM)
            nc.vector.tensor_sub(outT[0][:, sl(s)], hxi[0][0:UNITS, sl(s)],
                                 cT[0][:, sl(s)])
            nc.vector.tensor_mul(outT[0][:, sl(s)], uact[0][:, sl(s)],
                                 outT[0][:, sl(s)])
            nc.vector.tensor_add(outT[0][:, sl(s)], outT[0][:, sl(s)],
                                 cT[0][:, sl(s)])
            if s == 0:
                a1_block(7)
            if s == 1:
                nc.sync.dma_start(out_d[0, :, 0:1024], outT[0][:, 0:1024])
            if s == 3:
                nc.sync.dma_start(out_d[0, :, 1024:2048], outT[0][:, 1024:2048])

        # ---- b1 gconv1 close + gates (per-slab pipeline) ----
        ru1 = [pp.tile([128, 512], f32, tag="pA", name=f"ru1_{s}")
               for s in range(NS)]
        for s in range(NS):
            drain(x2T[1][:, sl(s)], psA1[s][0:F, :], s)
            nc.tensor.matmul(ru1[s][:], wru(0), xb[1][:, sl(s)],
                             start=True, stop=False)
            nc.tensor.matmul(ru1[s][:], wru(1), x1T[1][:, sl(s)],
                             start=False, stop=False)
            nc.tensor.matmul(ru1[s][:], wru(2), x2T[1][:, sl(s)],
                             start=False, stop=True)
            nc.scalar.activation(ract[1][:, sl(s)], ru1[s][0:UNITS, :],
                                 AF.Sigmoid)
            nc.vector.tensor_mul(rhT[1][:, sl(s)], ract[1][:, sl(s)],
                                 hxi[1][0:UNITS, sl(s)])
            nc.scalar.activation(uact[1][:, sl(s)], ru1[s][UNITS:128, :],
                                 AF.Sigmoid)
        psY1 = [pp.tile([128, 8, UNITS], f32, tag="pA", name=f"psY1_{i}")
                for i in range(4)]
        ymm(1, psY1[0:2], range(KC), 0)
        ymm(1, psY1[2:4], range(KC), 1)
        ynm_copies(1, psY1[0:2], psY1[2:4])

        # ---- b1 gconv2 streams (ring B after psC0 tanh-drains) ----
        psC1 = [pp.tile([UNITS, 512], f32, tag="pB", name=f"psC1_{s}")
                for s in range(NS)]
        for s in range(NS):
            nc.tensor.matmul(psC1[s][:], wc0I, hxi[1][UNITS:F, sl(s)],
                             start=True, stop=False)
            nc.tensor.matmul(psC1[s][:], wc0R, rhT[1][:, sl(s)],
                             start=False, stop=False)
        for kp in range(KP):
            for s in range(NS):
                dr_mm(psC1[s], y1nm[1][:, 2 * kp:2 * kp + 2, :], s2[kp], s,
                      False, False)
        for s in range(NS):
            for kp in range(KP):
                dr_mm(psC1[s], y2nm[1][:, 2 * kp:2 * kp + 2, :], a2[1][kp], s,
                      False, kp == KP - 1)
            nc.scalar.activation(cT[1][:, sl(s)], psC1[s][:],
                                 AF.Tanh, scale=1.0 / LAM)
            nc.vector.tensor_sub(outT[1][:, sl(s)], hxi[1][0:UNITS, sl(s)],
                                 cT[1][:, sl(s)])
            nc.vector.tensor_mul(outT[1][:, sl(s)], uact[1][:, sl(s)],
                                 outT[1][:, sl(s)])
            nc.vector.tensor_add(outT[1][:, sl(s)], outT[1][:, sl(s)],
                                 cT[1][:, sl(s)])
            if s == 1:
                nc.sync.dma_start(out_d[1, :, 0:1024], outT[1][:, 0:1024])
            if s == 3:
                nc.sync.dma_start(out_d[1, :, 1024:2048], outT[1][:, 1024:2048])

    nc.compile()
    _CACHE["nc"] = nc
    return nc


def _prep_host(inputs, hx, adp, support_rows, support_cols, support_vals,
               W_ru, W_c):
    xcat = np.concatenate(
        [inputs.reshape(B, N, D_IN), hx.reshape(B, N, UNITS)], axis=2)
    xcat = np.ascontiguousarray(xcat, dtype=np.float32)

    S = np.zeros((N, N), np.float32)
    np.add.at(S, (support_rows, support_cols), support_vals)
    # s2[kp, p, j, n] = S[n, 128*(2kp+j)+p] * 256
    s2 = np.ascontiguousarray(
        (S.T * S_SCALE).reshape(KP, 2, 128, N).transpose(0, 2, 1, 3)
    ).astype(FP8)

    # adp2[b, kp, p, j, n] = adp[b, n, 128*(2kp+j)+p] * 32768
    adp2 = np.ascontiguousarray(
        (adp.transpose(0, 2, 1) * A_SCALE).reshape(B, KP, 2, 128, N)
        .transpose(0, 1, 3, 2, 4)
    ).astype(FP8)

    # xnm[p, b, k, f] = x[b, 128k+p, f], feature-padded to FPAD
    xnm = np.zeros((B, 128, KC, FPAD), FP8)
    xnm[:, :, :, 0:F] = xcat.reshape(B, KC, 128, F).transpose(0, 2, 1, 3)
    xT = xcat.transpose(0, 2, 1)  # (B, F, N) feature-major
    xbh = np.ascontiguousarray(xT).astype(BF16)
    hxih = np.concatenate([xT[:, D_IN:F, :], xT[:, 0:D_IN, :]], axis=1)
    hxih = np.ascontiguousarray(hxih).astype(BF16)

    wru = np.ascontiguousarray(W_ru.reshape(F, 3, 2 * UNITS)).astype(np.float32)
    wru[:, 1, :] /= S_SCALE
    wru[:, 2, :] /= A_SCALE
    wc = W_c.reshape(F, 3, UNITS).astype(np.float32)
    wc0L = wc[:, 0, :] * LAM
    wcy = np.stack(
        [wc[:, 1, :] * (LAM / S_SCALE), wc[:, 2, :] * (LAM / A_SCALE)],
        axis=1)  # [F, 2, UNITS], rows = [inp(2) | hx(64)] feature order

    # Reorder wcy/wc0 rows to the hxi layout: rows 0:64 = hx-part rows,
    # rows 64:66 = inp-part rows.
    perm = np.concatenate([np.arange(D_IN, F), np.arange(0, D_IN)])
    wblob = np.zeros((F, 576), np.float32)
    wblob[:, 0:384] = wru.reshape(F, 384)
    wblob[:, 384:512] = wcy[perm].reshape(F, 128)
    wblob[:, 512:576] = wc0L[perm]

    shared = {"sT": s2, "wblob": wblob.astype(BF16)}
    in_maps = []
    for c in range(N_CORES):
        lo, hi = c * B_LOC, (c + 1) * B_LOC
        in_maps.append({
            "adpT": np.ascontiguousarray(adp2[lo:hi]),
            "xnm": np.ascontiguousarray(xnm[lo:hi].transpose(1, 0, 2, 3)),
            "xb": np.ascontiguousarray(xbh[lo:hi]),
            "hxi": np.ascontiguousarray(hxih[lo:hi]),
            **shared,
        })
    return in_maps


def kernel(inputs, hx, adp, support_rows, support_cols, support_vals,
           W_ru, W_c, time_axis=None):
    from concourse.bass_utils import run_bass_kernel_spmd

    inputs = np.asarray(inputs, dtype=np.float32)
    hx = np.asarray(hx, dtype=np.float32)
    adp = np.asarray(adp, dtype=np.float32)
    support_rows = np.asarray(support_rows)
    support_cols = np.asarray(support_cols)
    support_vals = np.asarray(support_vals, dtype=np.float32)
    W_ru = np.asarray(W_ru, dtype=np.float32)
    W_c = np.asarray(W_c, dtype=np.float32)

    nc = _build()
    in_maps = _prep_host(inputs, hx, adp, support_rows, support_cols,
                         support_vals, W_ru, W_c)

    res = run_bass_kernel_spmd(nc, in_maps, core_ids=list(range(N_CORES)),
                               trace=False)
    _CACHE["last_result"] = res

    out = np.empty((B, N * UNITS), np.float32)
    for c in range(N_CORES):
        outT = np.asarray(res.results[c]["outT"], dtype=np.float32)
        for i in range(B_LOC):
            out[c * B_LOC + i] = np.ascontiguousarray(
                outT[i].T).reshape(N * UNITS)
    return out
